# revision 2
# baseline (speedup 1.0000x reference)
# Trainium2 Bass kernel v2 for nn_ContrastiveReact.
#
# Per core (P sharded 8 x 6272): compute dots = em_tile.T @ phat (fp16 PE) and
# reduce max over the P shard per row. PSUM (8 banks) is split into four
# static 1024-wide slots: copyA/copyB/pairA/pairB. Per 128-row tile the 6272
# columns become 7 fills:
#   c0,c1,c2 (976 each) -> copy slots, ScalarE relocates them to SBUF fp32
#   p0,p1,p2 (976 each) -> pair slots, VectorE tensor_tensor_reduce(max,max)
#       pairs each PSUM fill against the matching relocated fill: 2 candidates
#       per cycle with a fused max accumulator (one acc col per fill)
#   e (416) -> copyB slot, ScalarE computes exp(S*cos - S*C) with a fused sum
#       accumulator (LSE surrogate for max; exact winner is almost never here
#       and ln(Neff)/S < 0.001 in cos units when it is)
# Static slots mean a fill only ever WAR-waits on the slot's previous consumer
# of the same type, so no producer->consumer->producer cycle exceeds the
# per-engine spend (~3.7us/tile); the kernel runs spend-bound.
# Host: combines the 3 pair maxima + LSE col per tile, then the tiny
# segmented softmax loss (same as baseline).
import tempfile

import numpy as np

import concourse.bacc as bacc
import concourse.bass as bass
import concourse.dve_ops as dve_ops
import concourse.mybir as mybir
import concourse.tile as tile
from concourse.bass_utils import run_bass_kernel_spmd
from concourse.dve_spec import C0, Spec, Src0, Src1, lower, maxx
from concourse.dve_uop import DveOpSpec

# Problem constants (hardcoded per harness contract).
B, NPOS, NNEG, P, D = 32, 16, 128, 50000, 128
NUM_GROUPS = 8
N_CORES = 8
ROWS = B * (NPOS + NNEG)          # 4608
RT = ROWS // 128                  # 36 row tiles
PC = 6272                         # padded P per core (8 * 6272 = 50176)
NEG_INIT = -1e30

CW = 944                          # copy/pair fill width
EW = PC - 6 * CW                  # exp fill width (416)
S_LSE = 300.0
C_LSE = 0.55

# PSUM slot bases (fp32 elements): copyA, copyB, pairA, pairB
CSLOT = (0, 1024)
PSLOT = (2048, 3072)

_CACHE = {}


def _ref_max_max(in0, in1, c0, c1, c2):
    b = np.maximum(in0.astype(np.float32), in1.astype(np.float32))
    acc = np.maximum(c0, b.reshape(b.shape[0], -1).max(axis=-1, keepdims=True))
    return b, acc


def register_max_max():
    """Custom DVE op: out = max(in0, in1); accum = max(s0, max(out)).
    (The native TENSOR_TENSOR_REDUCE opcode wedges the exec unit on this
    runtime, so the same dataflow is authored as a custom DVE op.)"""
    name = "TENSOR_MAX_MAX_ANT"
    for op in dve_ops.OPS:
        if op.name == name:
            return op
    spec = Spec(body=maxx(Src0, Src1), accum=maxx, accum_init=C0,
                reference=_ref_max_max)
    row = dve_ops._CUSTOM_DVE_ROW_BASE + len(dve_ops.OPS)
    assert row < 0x20
    shas = {}
    for ver in ("v3",):
        tmp = DveOpSpec(name=name, opcode=row, uops=lower(spec, ver=ver),
                        rd1_en=True)
        shas[ver] = tmp.sha(ver)
    op = dve_ops.DveOp(name, spec, subdim=False, uops_sha=shas)
    dve_ops.OPS.append(op)
    dve_ops._SUB_OPCODE_FOR_NAME[name] = row
    dve_ops.CUSTOM_DVE_SPECS[name] = spec
    return op


def _chunks(base, width):
    """Split [base, base+width) into <=512 pieces not crossing 512 banks."""
    out = []
    o = base
    end = base + width
    while o < end:
        w = min(512 - (o % 512), end - o)
        out.append((o, w))
        o += w
    return out


def build_nc():
    mm_op = register_max_max()
    nc = bacc.Bacc()
    ph = nc.dram_tensor("ph", [128, PC], mybir.dt.float8e4, kind="ExternalInput")
    em = nc.dram_tensor("em", [128, ROWS], mybir.dt.float8e4, kind="ExternalInput")
    sc = nc.dram_tensor("sc", [128, RT], mybir.dt.float32, kind="ExternalInput")
    # per tile: 3 pair-max accs + 1 exp-sum acc
    out = nc.dram_tensor("out", [128, 4 * RT], mybir.dt.float32,
                         kind="ExternalOutput")

    with tile.TileContext(nc) as tc:
        with (
            tc.tile_pool(name="singles", bufs=1) as singles,
            tc.tile_pool(name="cp", bufs=2) as cps,
            tc.tile_pool(name="scr", bufs=1) as scrp,
            tc.tile_pool(name="psr", bufs=1, space="PSUM") as pp,
        ):
            em_sb = singles.tile([128, ROWS], mybir.dt.float8e4)
            ph_sb = singles.tile([128, PC], mybir.dt.float8e4)
            sc_sb = singles.tile([128, RT], mybir.dt.float32)
            # interleave so tile 0's operands land first
            nc.sync.dma_start(out=em_sb[:, 0:128], in_=em[:, 0:128])
            cuts = [0, 2 * CW, 4 * CW, PC]
            for s in range(len(cuts) - 1):
                nc.sync.dma_start(out=ph_sb[:, cuts[s]:cuts[s + 1]],
                                  in_=ph[:, cuts[s]:cuts[s + 1]])
            nc.sync.dma_start(out=em_sb[:, 128:2048], in_=em[:, 128:2048])
            nc.sync.dma_start(out=sc_sb[:, :], in_=sc[:, :])
            nc.sync.dma_start(out=em_sb[:, 2048:ROWS], in_=em[:, 2048:ROWS])
            out_sb = singles.tile([128, 4 * RT], mybir.dt.float32)
            ring = pp.tile([128, 4096], mybir.dt.float32)
            pair_scr = scrp.tile([128, CW], mybir.dt.float32, tag="pscr")
            exp_scr = scrp.tile([128, EW], mybir.dt.float16, tag="escr")
            bias_sb = scrp.tile([128, 1], mybir.dt.float32, tag="bias")
            nc.vector.memset(bias_sb, -S_LSE * C_LSE)

            for t in range(RT):
                lhsT = em_sb[:, t * 128:(t + 1) * 128]
                cp = cps.tile([128, 3 * CW], mybir.dt.float32, tag="cp", bufs=2)

                def emit_exp(t=t, lhsT=lhsT):
                    eslot = CSLOT[(4 * t + 3) % 2]
                    ecol = 6 * CW
                    for (o, w) in _chunks(eslot, EW):
                        nc.tensor.matmul(ring[:, o:o + w], lhsT,
                                         ph_sb[:, ecol + o - eslot:
                                               ecol + o - eslot + w],
                                         start=True, stop=True)
                    nc.scalar.activation(
                        out=exp_scr[:, 0:EW], in_=ring[:, eslot:eslot + EW],
                        func=mybir.ActivationFunctionType.Exp,
                        bias=bias_sb[:, 0:1], scale=sc_sb[:, t:t + 1],
                        accum_out=out_sb[:, 4 * t + 3:4 * t + 4])

                col = 0
                for k in range(3):
                    if t == RT - 1 and k == 2:
                        # keep the last tile's exp off the critical tail
                        emit_exp()
                    cslot = CSLOT[(4 * t + k) % 2]
                    pslot = PSLOT[(3 * t + k) % 2]
                    # copy fill
                    for (o, w) in _chunks(cslot, CW):
                        nc.tensor.matmul(ring[:, o:o + w], lhsT,
                                         ph_sb[:, col + o - cslot:
                                               col + o - cslot + w],
                                         start=True, stop=True)
                    nc.scalar.copy(out=cp[:, k * CW:(k + 1) * CW],
                                   in_=ring[:, cslot:cslot + CW])
                    col += CW
                    # pair fill
                    for (o, w) in _chunks(pslot, CW):
                        nc.tensor.matmul(ring[:, o:o + w], lhsT,
                                         ph_sb[:, col + o - pslot:
                                               col + o - pslot + w],
                                         start=True, stop=True)
                    nc.vector._custom_dve(
                        mm_op, out=pair_scr[:, 0:CW],
                        in0=ring[:, pslot:pslot + CW],
                        in1=cp[:, k * CW:(k + 1) * CW],
                        s0=NEG_INIT,
                        accum_out=out_sb[:, 4 * t + k:4 * t + k + 1])
                    col += CW
                # exp fill (4th use of the copy-slot rotation)
                if t != RT - 1:
                    emit_exp()

                # drain results early in quarters so the final DMA is tiny
                if t in (8, 17, 26, 33):
                    lo = {8: 0, 17: 36, 26: 72, 33: 108}[t]
                    hi = lo + 36 if t != 33 else 136
                    nc.sync.dma_start(out=out[:, lo:hi],
                                      in_=out_sb[:, lo:hi])

            nc.sync.dma_start(out=out[:, 136:], in_=out_sb[:, 136:])
    nc.compile()
    return nc


def _prep(purch_embeddings, pos_embs, neg_embs):
    purch = np.asarray(purch_embeddings, dtype=np.float32)
    pos = np.asarray(pos_embs, dtype=np.float32)
    neg = np.asarray(neg_embs, dtype=np.float32)

    pnorm = np.sqrt((purch.astype(np.float64) ** 2).sum(axis=1))
    phat = purch / np.maximum(pnorm, 1e-8)[:, None]
    import ml_dtypes
    phatT = np.zeros((128, N_CORES * PC), dtype=ml_dtypes.float8_e4m3)
    phatT[:, :P] = phat.T.astype(ml_dtypes.float8_e4m3)
    shards = [np.ascontiguousarray(phatT[:, c * PC:(c + 1) * PC])
              for c in range(N_CORES)]

    embs = np.concatenate(
        [pos.reshape(B * NPOS, D), neg.reshape(B * NNEG, D)], axis=0)
    enorm = np.sqrt((embs.astype(np.float64) ** 2).sum(axis=1))
    import ml_dtypes
    embsT = np.ascontiguousarray(embs.T.astype(ml_dtypes.float8_e4m3))
    # per-row activation scale S_LSE / |em|, laid out [128, RT]
    scv = (S_LSE / np.maximum(enorm, 1e-8)).astype(np.float32)
    sc = np.ascontiguousarray(scv.reshape(RT, 128).T)
    return shards, embsT, sc, enorm


def run_device(shards, embsT, sc, trace=False):
    if "nc" not in _CACHE:
        _CACHE["nc"] = build_nc()
    nc = _CACHE["nc"]
    in_maps = [{"ph": shards[c], "em": embsT, "sc": sc}
               for c in range(N_CORES)]
    kwargs = {}
    if trace:
        kwargs = dict(trace=True, tmpdir=tempfile.mkdtemp(prefix="ctr_"))
    return run_bass_kernel_spmd(nc, in_maps, core_ids=list(range(N_CORES)),
                                **kwargs)


def _finish(results, enorm, cost_pos, cost_neg, neg_seg_ids):
    percore = []
    for r in results:
        o = r["out"].astype(np.float64)          # [128, 4*RT]
        M = np.empty(ROWS)
        for t in range(RT):
            rows = slice(t * 128, (t + 1) * 128)
            en = enorm[rows]
            mp = o[:, 4 * t:4 * t + 3].max(axis=1)
            cos = mp / np.maximum(en, 1e-8)
            S = o[:, 4 * t + 3]
            with np.errstate(divide="ignore"):
                cos_exp = np.where(
                    S > 0, C_LSE + np.log(np.maximum(S, 1e-300)) / S_LSE,
                    -np.inf)
            M[rows] = np.maximum(cos, cos_exp) * en
        percore.append(M)
    Mdot = np.stack(percore).max(axis=0)          # [ROWS] max dot over all P

    cos_max = Mdot / np.maximum(enorm, 1e-8)
    min_dist = 1.0 - cos_max
    pos_min = min_dist[:B * NPOS].reshape(B, NPOS)
    neg_min = min_dist[B * NPOS:].reshape(B, NNEG)

    cost_pos = np.asarray(cost_pos, dtype=np.float64)
    cost_neg = np.asarray(cost_neg, dtype=np.float64)
    seg = np.asarray(neg_seg_ids).astype(np.int64)

    positive_value = pos_min.sum(axis=1) + cost_pos
    seg_sum = np.zeros((B, NUM_GROUPS), dtype=np.float64)
    np.add.at(seg_sum, (np.arange(B)[:, None], seg), neg_min)
    negative_values = seg_sum + cost_neg

    num = np.exp(-positive_value)
    den = np.exp(-negative_values).sum(axis=1)
    losses = -np.log(num / (num + den))
    return np.array(losses.mean(), dtype=np.float32)


def kernel(purch_embeddings, pos_embs, neg_embs, cost_pos, cost_neg,
           neg_seg_ids):
    shards, embsT, sc, enorm = _prep(purch_embeddings, pos_embs, neg_embs)
    results = run_device(shards, embsT, sc, trace=False)
    return _finish(results.results, enorm, cost_pos, cost_neg, neg_seg_ids)
